# revision 8
# baseline (speedup 1.0000x reference)
import sys

sys.path.insert(0, "/opt/trn_rl_repo")

import numpy as np
import ml_dtypes

import concourse.bass as bass
from concourse import bacc
import concourse.mybir as mybir
import concourse.tile as tile
from concourse.bass_utils import run_bass_kernel_spmd

# Problem constants (nn_ConvLSTMAutoencoder: B=128, T=100, F=64, hid [16,32])
B_TOT, T, F = 128, 100, 64
NCORES = 8
B = B_TOT // NCORES          # 16 batch per core (pure data parallelism)
SEG = F + 2                  # spatial row with 1 zero pad col each side
C0, C1 = 16, 32

F32 = mybir.dt.float32
BF16 = mybir.dt.bfloat16
NP_BF16 = ml_dtypes.bfloat16

Tanh = mybir.ActivationFunctionType.Tanh
MULT = mybir.AluOpType.mult
ADD = mybir.AluOpType.add

NB = 8                       # batches per matmul (8*64 = 512 = psum bank cap)

# ---------------------------------------------------------------------------
# Depth-2 software pipeline: the first layer of each phase (enc0 / dec0)
# runs TWO steps ahead of the second layer (enc1 / dec1), with ping-pong
# arena slots for its hidden state, so the per-iteration critical cycle
# contains only ONE cell's chain (matmuls -> tanh -> gate math -> tanh ->
# h) instead of two chained cells.
#
# Cell math (tanh-trick): i/f/o weight+bias pre-halved -> one Tanh ACT
# gives th=2sig-1 for i/f/o and tanh for g; DVE fixup 0.5*th+0.5 restores
# sigmoids; then u=sig_i*tg, v=sig_f*c, c'=u+v, tc=tanh(c'), h=sig_o*tc.
# Gate columns: i@0:C f@32:32+C o@64:64+C g@96:96+C (32-aligned bases).
# ---------------------------------------------------------------------------


def _taps(nc, zt, wt, rhs_rows, arena):
    for d in range(3):
        for nb in range(0, B, NB):
            nc.tensor.matmul(
                zt[:, nb:nb + NB, :],
                wt[:, d, :],
                arena[rhs_rows, nb:nb + NB, d:d + F],
                start=(d == 0),
                stop=(d == 2),
            )


def _stages(nc, wpool, z, C, M, bvec, cc, h_of, offload=False):
    th = wpool.tile([M, B, F], BF16, tag="th", name="th")
    tgt = wpool.tile([C, B, F], BF16, tag="tg", name="tgt")
    uv = wpool.tile([C, 2, B, F], BF16, tag="uv", name="uv")
    tc = wpool.tile([64 + C, B, F], BF16, tag="tc", name="tc")
    fixrows = 64 + C

    def act_th():
        nc.scalar.activation(th[:], z[0:M], Tanh, bias=bvec[0:M])

    def dve_tg():
        nc.vector.tensor_scalar(tgt[:], th[96:96 + C], 1.0, 0.0, MULT, ADD)

    def dve_fix():
        eng = nc.gpsimd if offload else nc.vector
        eng.tensor_scalar(th[0:fixrows], th[0:fixrows], 0.5, 0.5,
                          MULT, ADD)

    def dve_u():
        eng = nc.gpsimd if offload else nc.vector
        eng.tensor_tensor(uv[:, 0], th[0:C], tgt[:], MULT)

    def dve_v():
        nc.vector.tensor_tensor(uv[:, 1], th[32:32 + C], cc[32:32 + C], MULT)

    def dve_add():
        nc.vector.tensor_tensor(cc[32:32 + C], uv[:, 0], uv[:, 1], ADD)

    def act_tc():
        nc.scalar.activation(tc[64:64 + C], cc[32:32 + C], Tanh)

    def dve_h():
        nc.vector.tensor_tensor(h_of(), th[64:64 + C], tc[64:64 + C], MULT)

    return (act_th, dve_tg, dve_fix, dve_u, dve_v, dve_add, act_tc, dve_h,
            th, uv)


def _emit_pair(cellA, cellB, warm=None):
    """A is the chain cell (its H gates the next iteration); B is the
    two-ahead lookahead cell with ~2 cycles of slack."""
    aTh, aTg, aFix, aU, aV, aAdd, aTc, aH, aThT, aUvT = cellA
    if cellB is None:
        aTh(); aTg(); aFix(); aU(); aV(); aAdd(); aTc(); aH()
        return
    bTh, bTg, bFix, bU, bV, bAdd, bTc, bH, bThT, bUvT = cellB
    aTh()                  # ACT
    bTh()                  # ACT
    aTg(); aFix()          # DVE
    if warm:
        warm(aThT[0:1, 0, 0:1])
    aU(); aV()             # DVE chain of A
    if warm:
        warm(aUvT[0:1, 1, 0, 0:1])
    aAdd()
    aTc()                  # ACT
    bTg(); bFix()          # DVE
    bU()
    if warm:
        warm(bUvT[0:1, 0, 0, 0:1])
    bV()
    if warm:
        warm(bUvT[0:1, 1, 0, 0:1])
    aH()                   # DVE -- unblocks next iteration's matmuls
    bAdd()
    if warm:
        warm(bThT[0:1, 0, 0:1])
    bTc()                  # ACT
    bH()                   # DVE


def build_program():
    nc = bacc.Bacc(None)

    x_pad = nc.declare_dram_parameter("x_pad", [T, B, SEG], BF16, isOutput=False)
    we0a = nc.declare_dram_parameter("we0a", [80, 3, 112], BF16, isOutput=False)
    we0b = nc.declare_dram_parameter("we0b", [80, 3, 112], BF16, isOutput=False)
    we1a = nc.declare_dram_parameter("we1a", [80, 3, 128], BF16, isOutput=False)
    we1b = nc.declare_dram_parameter("we1b", [80, 3, 128], BF16, isOutput=False)
    wd0a = nc.declare_dram_parameter("wd0a", [128, 3, 128], BF16, isOutput=False)
    wd0b = nc.declare_dram_parameter("wd0b", [128, 3, 128], BF16, isOutput=False)
    wd1a = nc.declare_dram_parameter("wd1a", [96, 3, 112], BF16, isOutput=False)
    wd1b = nc.declare_dram_parameter("wd1b", [96, 3, 112], BF16, isOutput=False)
    fcv = nc.declare_dram_parameter("fcv", [17, 1], BF16, isOutput=False)
    b0 = nc.declare_dram_parameter("b0", [112, 1], F32, isOutput=False)
    b1 = nc.declare_dram_parameter("b1", [128, 1], F32, isOutput=False)
    bd0 = nc.declare_dram_parameter("bd0", [128, 1], F32, isOutput=False)
    bd1 = nc.declare_dram_parameter("bd1", [112, 1], F32, isOutput=False)
    out = nc.declare_dram_parameter("out", [B, T, F], BF16, isOutput=True)

    with tile.TileContext(nc) as tc:
        with (
            tc.tile_pool(name="const", bufs=1) as cpool,
            tc.tile_pool(name="state", bufs=1) as spool,
            tc.tile_pool(name="work", bufs=2) as wpool,
            tc.tile_pool(name="zp", bufs=2, space="PSUM") as zpool,
            tc.tile_pool(name="fcp", bufs=1, space="PSUM") as fcpool,
        ):
            we0at = cpool.tile([80, 3, 112], BF16)
            we0bt = cpool.tile([80, 3, 112], BF16)
            we1at = cpool.tile([80, 3, 128], BF16)
            we1bt = cpool.tile([80, 3, 128], BF16)
            wd0at = cpool.tile([128, 3, 128], BF16)
            wd0bt = cpool.tile([128, 3, 128], BF16)
            wd1at = cpool.tile([96, 3, 112], BF16)
            wd1bt = cpool.tile([96, 3, 112], BF16)
            fcvt = cpool.tile([17, 1], BF16)
            warml = cpool.tile([1, 1], BF16)
            b0t = cpool.tile([112, 1], F32)
            b1t = cpool.tile([128, 1], F32)
            bd0t = cpool.tile([128, 1], F32)
            bd1t = cpool.tile([112, 1], F32)
            for dst, dsrc in [(we0at, we0a), (we0bt, we0b), (we1at, we1a),
                              (we1bt, we1b), (wd0at, wd0a), (wd0bt, wd0b),
                              (wd1at, wd1a), (wd1bt, wd1b),
                              (fcvt, fcv), (b0t, b0),
                              (b1t, b1), (bd0t, bd0), (bd1t, bd1)]:
                nc.sync.dma_start(dst[:], dsrc[:])
            nc.vector.memset(warml[:], 0.0)

            def warm(rhs_ap):
                wp = fcpool.tile([1, 1], F32, tag="warm", name="wp")
                nc.tensor.matmul(wp[:], warml[:], rhs_ap, start=True,
                                 stop=True)

            # Encoder arena: 0:32 H1 | 32:48 H0 slotA | 48 xA | 49 xB |
            #                64:80 H0 slotB          (both rhs = rows 0:80)
            # Decoder arena: 0:16 Hd1 | 16 ones | 32:64 Hd0 slotA |
            #                64:96 Hd0 slotB | 96:128 e2
            arena_e = spool.tile([80, B, SEG], BF16)
            arena_d = spool.tile([128, B, SEG], BF16)
            seq = spool.tile([128, (T + 3) // 4, B, F], BF16)
            nc.vector.memset(arena_e[:], 0.0)
            nc.vector.memset(arena_d[:], 0.0)
            nc.vector.memset(arena_d[0:17], 1.0)   # row 16 stays 1.0 (fc bias)
            nc.vector.memset(arena_d[0:16], 0.0)

            # Persistent cell state rows 32:32+C (carried enc1->dec0 and
            # enc0->dec1 across the phase switch; no copies needed).
            cc1 = spool.tile([64, B, F], BF16)
            cc0 = spool.tile([64, B, F], BF16)
            nc.vector.memset(cc1[32:64], 0.0)
            nc.vector.memset(cc0[32:48], 0.0)

            E0SLOT = (slice(32, 48), slice(64, 80))   # h0 slot by t%2
            D0SLOT = (slice(32, 64), slice(64, 96))   # hd0 slot by t%2

            def cell_e1(z):
                return _stages(nc, wpool, z, C1, 128, b1t, cc1,
                               lambda: arena_e[0:32, :, 1:1 + F])

            def cell_e0(z, t):
                s = E0SLOT[t % 2]
                return _stages(nc, wpool, z, C0, 112, b0t, cc0,
                               lambda: arena_e[s, :, 1:1 + F], offload=True)

            def cell_d0(z, t):
                s = D0SLOT[t % 2]
                return _stages(nc, wpool, z, C1, 128, bd0t, cc1,
                               lambda: arena_d[s, :, 1:1 + F], offload=True)

            def cell_d1(z):
                return _stages(nc, wpool, z, C0, 112, bd1t, cc0,
                               lambda: arena_d[0:16, :, 1:1 + F])

            def z_e0(t):
                z0 = zpool.tile([112, B, F], F32, tag="z", name="z0")
                _taps(nc, z0, we0at if t % 2 == 0 else we0bt,
                      slice(0, 80), arena_e)
                return z0

            def z_e1(t):
                z1 = zpool.tile([128, B, F], F32, tag="z", name="z1")
                _taps(nc, z1, we1at if t % 2 == 0 else we1bt,
                      slice(0, 80), arena_e)
                return z1

            def z_d0(t):
                zd = zpool.tile([128, B, F], F32, tag="z", name="zd0")
                _taps(nc, zd, wd0at if t % 2 == 0 else wd0bt,
                      slice(0, 128), arena_d)
                return zd

            def z_d1(t):
                zd = zpool.tile([112, B, F], F32, tag="z", name="zd1")
                _taps(nc, zd, wd1at if t % 2 == 0 else wd1bt,
                      slice(0, 96), arena_d)
                return zd

            # ---------------- encoder ----------------
            # prologue: enc0(0) and enc0(1) solo; x(0)@48, x(1)@49
            nc.sync.dma_start(arena_e[48:49, :, :], x_pad[0:1, :, :])
            nc.sync.dma_start(arena_e[49:50, :, :], x_pad[1:2, :, :])
            _emit_pair(cell_e0(z_e0(0), 0), None)
            if T > 2:
                nc.sync.dma_start(arena_e[48:49, :, :], x_pad[2:3, :, :])
            _emit_pair(cell_e0(z_e0(1), 1), None)
            # loop iteration t: enc1(t) [chain] + enc0(t+2) [lookahead]
            for t in range(T):
                if t + 3 < T:
                    xrow = 48 + ((t + 3) % 2)
                    nc.sync.dma_start(arena_e[xrow:xrow + 1, :, :],
                                      x_pad[t + 3:t + 4, :, :])
                cellA = cell_e1(z_e1(t))
                cellB = None
                if t + 2 < T:
                    cellB = cell_e0(z_e0(t + 2), t + 2)
                _emit_pair(cellA, cellB, warm)
                r = (t % 4) * 32
                nc.sync.dma_start(seq[r:r + 32, t // 4],
                                  arena_e[0:32, :, 1:1 + F])

            # ---------------- decoder init ----------------
            # hd1(-1) = enc0 final h (slot (T-1)%2); hd0(-1) = enc1 final h
            nc.sync.dma_start(arena_d[0:16, :, :],
                              arena_e[E0SLOT[(T - 1) % 2], :, :])
            nc.sync.dma_start(arena_d[64:96, :, :], arena_e[0:32, :, :])

            def emit_fc(t):
                zfc = fcpool.tile([1, B, F], F32, tag="fc", name="zfc")
                for nb in range(0, B, NB):
                    nc.tensor.matmul(zfc[:, nb:nb + NB, :], fcvt[:],
                                     arena_d[0:17, nb:nb + NB, 1:1 + F],
                                     start=True, stop=True)
                ofc = wpool.tile([1, B, F], BF16, tag="ofc", name="ofc")
                nc.scalar.copy(ofc[:], zfc[:])
                nc.sync.dma_start(out[:, t, :], ofc[0:1, :, :])

            def load_e2(t):
                r = (t % 4) * 32
                nc.sync.dma_start(arena_d[96:128, :, 1:1 + F],
                                  seq[r:r + 32, t // 4])

            # prologue: dec0(0), dec0(1) solo.  dec0(0) reads hd0(-1)@slotB
            load_e2(0)
            _emit_pair(cell_d0(z_d0(0), 0), None)
            load_e2(1)
            _emit_pair(cell_d0(z_d0(1), 1), None)
            # loop iteration t: dec1(t) [chain] + dec0(t+2) [lookahead]
            for t in range(T):
                if t > 0:
                    emit_fc(t - 1)
                if t + 2 < T:
                    load_e2(t + 2)
                cellA = cell_d1(z_d1(t))
                cellB = None
                if t + 2 < T:
                    cellB = cell_d0(z_d0(t + 2), t + 2)
                _emit_pair(cellA, cellB, warm)

            emit_fc(T - 1)

    nc.finalize()
    return nc


def _prep_weights(w, b, Cin, C, row_map, M):
    """[4C, Cin, 3, 3] -> lhsT [len(row_map), 3, M], bias [M, 1]. Gate
    order i,f,o,g -> cols i@0 f@32 o@64 g@96; i/f/o scaled 0.5."""
    w3 = np.asarray(w, np.float32).reshape(4 * C, Cin, 3, 3)[:, :, :, 1]
    b = np.asarray(b, np.float32).reshape(4 * C)
    lhsT = np.zeros((len(row_map), 3, M), np.float32)
    bvec = np.zeros((M, 1), np.float32)
    for gi, col0 in enumerate((0, 32, 64, 96)):
        gate_s = 1.0 if gi == 3 else 0.5
        for j in range(C):
            oc = gi * C + j
            bvec[col0 + j, 0] = b[oc] * gate_s
            for r, ch in enumerate(row_map):
                if ch >= 0:
                    lhsT[r, :, col0 + j] = w3[oc, ch, :] * gate_s
    return np.ascontiguousarray(lhsT).astype(NP_BF16), bvec


_CACHE = {}


def kernel(x, enc_w0, enc_b0, enc_w1, enc_b1, dec_w0, dec_b0, dec_w1, dec_b1,
           fc_w, fc_b):
    if "nc" not in _CACHE:
        _CACHE["nc"] = build_program()
    nc = _CACHE["nc"]

    x = np.asarray(x, np.float32)
    ZR = [-1]
    # enc0(t): input ch = [x(0), h0(1:17)]
    #   even t: h0(t-1)@rows 64:80, x(t)@row 48
    #   odd  t: h0(t-1)@rows 32:48, x(t)@row 49
    we0a, b0 = _prep_weights(enc_w0, enc_b0, 1 + C0, C0,
                             ZR * 48 + [0] + ZR * 15 +
                             list(range(1, 17)), 112)
    we0b, _ = _prep_weights(enc_w0, enc_b0, 1 + C0, C0,
                            ZR * 32 + list(range(1, 17)) + ZR + [0] +
                            ZR * 30, 112)
    # enc1(t): input ch = [h0(0:16), h1(16:48)]; h1@rows 0:32,
    #   h0(t)@rows 32:48 (even t) or 64:80 (odd t)
    we1a, b1 = _prep_weights(enc_w1, enc_b1, C0 + C1, C1,
                             list(range(16, 48)) + list(range(16)) +
                             ZR * 32, 128)
    we1b, _ = _prep_weights(enc_w1, enc_b1, C0 + C1, C1,
                            list(range(16, 48)) + ZR * 32 +
                            list(range(16)), 128)
    # dec0(t): input ch = [e2(0:32), hd0(32:64)]; e2@rows 96:128,
    #   hd0(t-1)@rows 64:96 (even t) or 32:64 (odd t)
    wd0a, bd0 = _prep_weights(dec_w0, dec_b0, C1 + C1, C1,
                              ZR * 64 + list(range(32, 64)) +
                              list(range(32)), 128)
    wd0b, _ = _prep_weights(dec_w0, dec_b0, C1 + C1, C1,
                            ZR * 32 + list(range(32, 64)) + ZR * 32 +
                            list(range(32)), 128)
    # dec1(t): input ch = [hd0(0:32), hd1(32:48)]; hd1@rows 0:16,
    #   hd0(t)@rows 32:64 (even t) or 64:96 (odd t)
    wd1a, bd1 = _prep_weights(dec_w1, dec_b1, C1 + C0, C0,
                              list(range(32, 48)) + ZR * 16 +
                              list(range(32)) + ZR * 32, 112)
    wd1b, _ = _prep_weights(dec_w1, dec_b1, C1 + C0, C0,
                            list(range(32, 48)) + ZR * 16 + ZR * 32 +
                            list(range(32)), 112)
    fcv = np.concatenate(
        [np.asarray(fc_w, np.float32).reshape(C0),
         np.asarray(fc_b, np.float32).reshape(1)]).reshape(17, 1)
    fcv = np.ascontiguousarray(fcv).astype(NP_BF16)

    in_maps = []
    for core in range(NCORES):
        xs = x[core * B:(core + 1) * B]      # [B, T, F]
        xp = np.zeros((T, B, SEG), np.float32)
        xp[:, :, 1:1 + F] = xs.transpose(1, 0, 2)
        in_maps.append({
            "x_pad": xp.astype(NP_BF16),
            "we0a": we0a, "we0b": we0b, "we1a": we1a, "we1b": we1b,
            "wd0a": wd0a, "wd0b": wd0b, "wd1a": wd1a, "wd1b": wd1b,
            "fcv": fcv,
            "b0": b0, "b1": b1, "bd0": bd0, "bd1": bd1,
        })

    _CACHE["in_maps"] = in_maps
    res = run_bass_kernel_spmd(nc, in_maps, core_ids=list(range(NCORES)))
    outs = [res.results[i]["out"] for i in range(NCORES)]
    return np.concatenate(outs, axis=0).astype(np.float32)


if __name__ == "__main__":
    rng = np.random.default_rng(0)
    inputs = {
        "x": rng.standard_normal((B_TOT, T, F), dtype=np.float32),
        "enc_w0": rng.standard_normal((4 * C0, 1 + C0, 3, 3), dtype=np.float32) * 0.05,
        "enc_b0": np.zeros(4 * C0, np.float32),
        "enc_w1": rng.standard_normal((4 * C1, C0 + C1, 3, 3), dtype=np.float32) * 0.05,
        "enc_b1": np.zeros(4 * C1, np.float32),
        "dec_w0": rng.standard_normal((4 * C1, C1 + C1, 3, 3), dtype=np.float32) * 0.05,
        "dec_b0": np.zeros(4 * C1, np.float32),
        "dec_w1": rng.standard_normal((4 * C0, C1 + C0, 3, 3), dtype=np.float32) * 0.05,
        "dec_b1": np.zeros(4 * C0, np.float32),
        "fc_w": rng.standard_normal((1, C0, 1, 1), dtype=np.float32) * 0.05,
        "fc_b": np.zeros(1, np.float32),
    }
    out = kernel(**inputs)
    print("out", out.shape, out.dtype, np.abs(out).max())


# revision 9
# speedup vs baseline: 1.0001x; 1.0001x over previous
import sys

sys.path.insert(0, "/opt/trn_rl_repo")

import numpy as np
import ml_dtypes

import concourse.bass as bass
from concourse import bacc
import concourse.mybir as mybir
import concourse.tile as tile
from concourse.bass_utils import run_bass_kernel_spmd

# Problem constants (nn_ConvLSTMAutoencoder: B=128, T=100, F=64, hid [16,32])
B_TOT, T, F = 128, 100, 64
NCORES = 8
B = B_TOT // NCORES          # 16 batch per core (pure data parallelism)
SEG = F + 2                  # spatial row with 1 zero pad col each side
C0, C1 = 16, 32

F32 = mybir.dt.float32
BF16 = mybir.dt.bfloat16
NP_BF16 = ml_dtypes.bfloat16

Tanh = mybir.ActivationFunctionType.Tanh
MULT = mybir.AluOpType.mult
ADD = mybir.AluOpType.add

NB = 8                       # batches per matmul (8*64 = 512 = psum bank cap)

# ---------------------------------------------------------------------------
# Depth-2 software pipeline: the first layer of each phase (enc0 / dec0)
# runs TWO steps ahead of the second layer (enc1 / dec1), with ping-pong
# arena slots for its hidden state, so the per-iteration critical cycle
# contains only ONE cell's chain (matmuls -> tanh -> gate math -> tanh ->
# h) instead of two chained cells.
#
# Cell math (tanh-trick): i/f/o weight+bias pre-halved -> one Tanh ACT
# gives th=2sig-1 for i/f/o and tanh for g; DVE fixup 0.5*th+0.5 restores
# sigmoids; then u=sig_i*tg, v=sig_f*c, c'=u+v, tc=tanh(c'), h=sig_o*tc.
# Gate columns: i@0:C f@32:32+C o@64:64+C g@96:96+C (32-aligned bases).
# ---------------------------------------------------------------------------


def _taps(nc, zt, wt, rhs_rows, arena):
    for d in range(3):
        for nb in range(0, B, NB):
            nc.tensor.matmul(
                zt[:, nb:nb + NB, :],
                wt[:, d, :],
                arena[rhs_rows, nb:nb + NB, d:d + F],
                start=(d == 0),
                stop=(d == 2),
            )


def _stages(nc, wpool, z, C, M, bvec, cc, h_of, offload=False):
    th = wpool.tile([M, B, F], BF16, tag="th", name="th")
    tgt = wpool.tile([C, B, F], BF16, tag="tg", name="tgt")
    uv = wpool.tile([C, 2, B, F], BF16, tag="uv", name="uv")
    tc = wpool.tile([64 + C, B, F], BF16, tag="tc", name="tc")
    fixrows = 64 + C

    def act_th():
        nc.scalar.activation(th[:], z[0:M], Tanh, bias=bvec[0:M])

    def dve_tg():
        nc.vector.tensor_scalar(tgt[:], th[96:96 + C], 1.0, 0.0, MULT, ADD)

    def dve_fix():
        eng = nc.gpsimd if offload else nc.vector
        eng.tensor_scalar(th[0:fixrows], th[0:fixrows], 0.5, 0.5,
                          MULT, ADD)

    def dve_u():
        eng = nc.gpsimd if offload else nc.vector
        eng.tensor_tensor(uv[:, 0], th[0:C], tgt[:], MULT)

    def dve_v():
        nc.vector.tensor_tensor(uv[:, 1], th[32:32 + C], cc[32:32 + C], MULT)

    def dve_add():
        nc.vector.tensor_tensor(cc[32:32 + C], uv[:, 0], uv[:, 1], ADD)

    mk = wpool.tile([1, 1], BF16, tag="mk", name="mk")

    def act_tc():
        nc.scalar.activation(tc[64:64 + C], cc[32:32 + C], Tanh)
        nc.scalar.activation(mk[:], cc[32:33, 0:1, 0:1], Tanh)

    def dve_h():
        nc.vector.tensor_tensor(h_of(), th[64:64 + C], tc[64:64 + C], MULT)

    return (act_th, dve_tg, dve_fix, dve_u, dve_v, dve_add, act_tc, dve_h,
            th, uv, mk, h_of)


def _emit_pair(cellA, cellB, warm=None):
    """A is the chain cell (its H gates the next iteration); B is the
    two-ahead lookahead cell with ~2 cycles of slack."""
    (aTh, aTg, aFix, aU, aV, aAdd, aTc, aH, aThT, aUvT, aMk,
     aHof) = cellA
    if cellB is None:
        aTh(); aTg(); aFix(); aU(); aV(); aAdd(); aTc(); aH()
        return
    (bTh, bTg, bFix, bU, bV, bAdd, bTc, bH, bThT, bUvT, bMk,
     bHof) = cellB
    aTh()                  # ACT
    bTh()                  # ACT
    aTg(); aFix()          # DVE
    if warm:
        warm(aThT[0:1, 0, 0:1])
    aU(); aV()             # DVE chain of A
    if warm:
        warm(aUvT[0:1, 1, 0, 0:1])
    aAdd()
    aTc()                  # ACT
    if warm:
        warm(aMk[0:1, 0:1])
    bTg(); bFix()          # DVE
    bU()
    if warm:
        warm(bUvT[0:1, 0, 0, 0:1])
    bV()
    if warm:
        warm(bUvT[0:1, 1, 0, 0:1])
    aH()                   # DVE -- unblocks next iteration's matmuls
    if warm:
        warm(aHof()[0:1, 0:1, 0:1])
    bAdd()
    if warm:
        warm(bThT[0:1, 0, 0:1])
    bTc()                  # ACT
    bH()                   # DVE


def build_program():
    nc = bacc.Bacc(None)

    x_pad = nc.declare_dram_parameter("x_pad", [T, B, SEG], BF16, isOutput=False)
    we0a = nc.declare_dram_parameter("we0a", [80, 3, 112], BF16, isOutput=False)
    we0b = nc.declare_dram_parameter("we0b", [80, 3, 112], BF16, isOutput=False)
    we1a = nc.declare_dram_parameter("we1a", [80, 3, 128], BF16, isOutput=False)
    we1b = nc.declare_dram_parameter("we1b", [80, 3, 128], BF16, isOutput=False)
    wd0a = nc.declare_dram_parameter("wd0a", [128, 3, 128], BF16, isOutput=False)
    wd0b = nc.declare_dram_parameter("wd0b", [128, 3, 128], BF16, isOutput=False)
    wd1a = nc.declare_dram_parameter("wd1a", [96, 3, 112], BF16, isOutput=False)
    wd1b = nc.declare_dram_parameter("wd1b", [96, 3, 112], BF16, isOutput=False)
    fcv = nc.declare_dram_parameter("fcv", [17, 1], BF16, isOutput=False)
    b0 = nc.declare_dram_parameter("b0", [112, 1], F32, isOutput=False)
    b1 = nc.declare_dram_parameter("b1", [128, 1], F32, isOutput=False)
    bd0 = nc.declare_dram_parameter("bd0", [128, 1], F32, isOutput=False)
    bd1 = nc.declare_dram_parameter("bd1", [112, 1], F32, isOutput=False)
    out = nc.declare_dram_parameter("out", [B, T, F], BF16, isOutput=True)

    with tile.TileContext(nc) as tc:
        with (
            tc.tile_pool(name="const", bufs=1) as cpool,
            tc.tile_pool(name="state", bufs=1) as spool,
            tc.tile_pool(name="work", bufs=2) as wpool,
            tc.tile_pool(name="zp", bufs=2, space="PSUM") as zpool,
            tc.tile_pool(name="fcp", bufs=1, space="PSUM") as fcpool,
        ):
            we0at = cpool.tile([80, 3, 112], BF16)
            we0bt = cpool.tile([80, 3, 112], BF16)
            we1at = cpool.tile([80, 3, 128], BF16)
            we1bt = cpool.tile([80, 3, 128], BF16)
            wd0at = cpool.tile([128, 3, 128], BF16)
            wd0bt = cpool.tile([128, 3, 128], BF16)
            wd1at = cpool.tile([96, 3, 112], BF16)
            wd1bt = cpool.tile([96, 3, 112], BF16)
            fcvt = cpool.tile([17, 1], BF16)
            warml = cpool.tile([1, 1], BF16)
            b0t = cpool.tile([112, 1], F32)
            b1t = cpool.tile([128, 1], F32)
            bd0t = cpool.tile([128, 1], F32)
            bd1t = cpool.tile([112, 1], F32)
            for dst, dsrc in [(we0at, we0a), (we0bt, we0b), (we1at, we1a),
                              (we1bt, we1b), (wd0at, wd0a), (wd0bt, wd0b),
                              (wd1at, wd1a), (wd1bt, wd1b),
                              (fcvt, fcv), (b0t, b0),
                              (b1t, b1), (bd0t, bd0), (bd1t, bd1)]:
                nc.sync.dma_start(dst[:], dsrc[:])
            nc.vector.memset(warml[:], 0.0)

            def warm(rhs_ap):
                wp = fcpool.tile([1, 1], F32, tag="warm", name="wp")
                nc.tensor.matmul(wp[:], warml[:], rhs_ap, start=True,
                                 stop=True)

            # Encoder arena: 0:32 H1 | 32:48 H0 slotA | 48 xA | 49 xB |
            #                64:80 H0 slotB          (both rhs = rows 0:80)
            # Decoder arena: 0:16 Hd1 | 16 ones | 32:64 Hd0 slotA |
            #                64:96 Hd0 slotB | 96:128 e2
            arena_e = spool.tile([80, B, SEG], BF16)
            arena_d = spool.tile([128, B, SEG], BF16)
            seq = spool.tile([128, (T + 3) // 4, B, F], BF16)
            nc.vector.memset(arena_e[:], 0.0)
            nc.vector.memset(arena_d[:], 0.0)
            nc.vector.memset(arena_d[0:17], 1.0)   # row 16 stays 1.0 (fc bias)
            nc.vector.memset(arena_d[0:16], 0.0)

            # Persistent cell state rows 32:32+C (carried enc1->dec0 and
            # enc0->dec1 across the phase switch; no copies needed).
            cc1 = spool.tile([64, B, F], BF16)
            cc0 = spool.tile([64, B, F], BF16)
            nc.vector.memset(cc1[32:64], 0.0)
            nc.vector.memset(cc0[32:48], 0.0)

            E0SLOT = (slice(32, 48), slice(64, 80))   # h0 slot by t%2
            D0SLOT = (slice(32, 64), slice(64, 96))   # hd0 slot by t%2

            def cell_e1(z):
                return _stages(nc, wpool, z, C1, 128, b1t, cc1,
                               lambda: arena_e[0:32, :, 1:1 + F])

            def cell_e0(z, t):
                s = E0SLOT[t % 2]
                return _stages(nc, wpool, z, C0, 112, b0t, cc0,
                               lambda: arena_e[s, :, 1:1 + F], offload=True)

            def cell_d0(z, t):
                s = D0SLOT[t % 2]
                return _stages(nc, wpool, z, C1, 128, bd0t, cc1,
                               lambda: arena_d[s, :, 1:1 + F], offload=True)

            def cell_d1(z):
                return _stages(nc, wpool, z, C0, 112, bd1t, cc0,
                               lambda: arena_d[0:16, :, 1:1 + F])

            def z_e0(t):
                z0 = zpool.tile([112, B, F], F32, tag="z", name="z0")
                _taps(nc, z0, we0at if t % 2 == 0 else we0bt,
                      slice(0, 80), arena_e)
                return z0

            def z_e1(t):
                z1 = zpool.tile([128, B, F], F32, tag="z", name="z1")
                _taps(nc, z1, we1at if t % 2 == 0 else we1bt,
                      slice(0, 80), arena_e)
                return z1

            def z_d0(t):
                zd = zpool.tile([128, B, F], F32, tag="z", name="zd0")
                _taps(nc, zd, wd0at if t % 2 == 0 else wd0bt,
                      slice(0, 128), arena_d)
                return zd

            def z_d1(t):
                zd = zpool.tile([112, B, F], F32, tag="z", name="zd1")
                _taps(nc, zd, wd1at if t % 2 == 0 else wd1bt,
                      slice(0, 96), arena_d)
                return zd

            # ---------------- encoder ----------------
            # prologue: enc0(0) and enc0(1) solo; x(0)@48, x(1)@49
            nc.sync.dma_start(arena_e[48:49, :, :], x_pad[0:1, :, :])
            nc.sync.dma_start(arena_e[49:50, :, :], x_pad[1:2, :, :])
            _emit_pair(cell_e0(z_e0(0), 0), None)
            if T > 2:
                nc.sync.dma_start(arena_e[48:49, :, :], x_pad[2:3, :, :])
            _emit_pair(cell_e0(z_e0(1), 1), None)
            # loop iteration t: enc1(t) [chain] + enc0(t+2) [lookahead]
            for t in range(T):
                if t + 3 < T:
                    xrow = 48 + ((t + 3) % 2)
                    nc.sync.dma_start(arena_e[xrow:xrow + 1, :, :],
                                      x_pad[t + 3:t + 4, :, :])
                cellA = cell_e1(z_e1(t))
                cellB = None
                if t + 2 < T:
                    cellB = cell_e0(z_e0(t + 2), t + 2)
                _emit_pair(cellA, cellB, warm)
                r = (t % 4) * 32
                nc.sync.dma_start(seq[r:r + 32, t // 4],
                                  arena_e[0:32, :, 1:1 + F])

            # ---------------- decoder init ----------------
            # hd1(-1) = enc0 final h (slot (T-1)%2); hd0(-1) = enc1 final h
            nc.sync.dma_start(arena_d[0:16, :, :],
                              arena_e[E0SLOT[(T - 1) % 2], :, :])
            nc.sync.dma_start(arena_d[64:96, :, :], arena_e[0:32, :, :])

            def emit_fc(t):
                zfc = fcpool.tile([1, B, F], F32, tag="fc", name="zfc")
                for nb in range(0, B, NB):
                    nc.tensor.matmul(zfc[:, nb:nb + NB, :], fcvt[:],
                                     arena_d[0:17, nb:nb + NB, 1:1 + F],
                                     start=True, stop=True)
                ofc = wpool.tile([1, B, F], BF16, tag="ofc", name="ofc")
                nc.scalar.copy(ofc[:], zfc[:])
                nc.sync.dma_start(out[:, t, :], ofc[0:1, :, :])

            def load_e2(t):
                r = (t % 4) * 32
                nc.sync.dma_start(arena_d[96:128, :, 1:1 + F],
                                  seq[r:r + 32, t // 4])

            # prologue: dec0(0), dec0(1) solo.  dec0(0) reads hd0(-1)@slotB
            load_e2(0)
            _emit_pair(cell_d0(z_d0(0), 0), None)
            load_e2(1)
            _emit_pair(cell_d0(z_d0(1), 1), None)
            # loop iteration t: dec1(t) [chain] + dec0(t+2) [lookahead]
            for t in range(T):
                if t + 2 < T:
                    load_e2(t + 2)
                zd1_t = z_d1(t)
                if t > 0:
                    emit_fc(t - 1)
                cellA = cell_d1(zd1_t)
                cellB = None
                if t + 2 < T:
                    cellB = cell_d0(z_d0(t + 2), t + 2)
                _emit_pair(cellA, cellB, warm)

            emit_fc(T - 1)

    nc.finalize()
    return nc


def _prep_weights(w, b, Cin, C, row_map, M):
    """[4C, Cin, 3, 3] -> lhsT [len(row_map), 3, M], bias [M, 1]. Gate
    order i,f,o,g -> cols i@0 f@32 o@64 g@96; i/f/o scaled 0.5."""
    w3 = np.asarray(w, np.float32).reshape(4 * C, Cin, 3, 3)[:, :, :, 1]
    b = np.asarray(b, np.float32).reshape(4 * C)
    lhsT = np.zeros((len(row_map), 3, M), np.float32)
    bvec = np.zeros((M, 1), np.float32)
    for gi, col0 in enumerate((0, 32, 64, 96)):
        gate_s = 1.0 if gi == 3 else 0.5
        for j in range(C):
            oc = gi * C + j
            bvec[col0 + j, 0] = b[oc] * gate_s
            for r, ch in enumerate(row_map):
                if ch >= 0:
                    lhsT[r, :, col0 + j] = w3[oc, ch, :] * gate_s
    return np.ascontiguousarray(lhsT).astype(NP_BF16), bvec


_CACHE = {}


def kernel(x, enc_w0, enc_b0, enc_w1, enc_b1, dec_w0, dec_b0, dec_w1, dec_b1,
           fc_w, fc_b):
    if "nc" not in _CACHE:
        _CACHE["nc"] = build_program()
    nc = _CACHE["nc"]

    x = np.asarray(x, np.float32)
    ZR = [-1]
    # enc0(t): input ch = [x(0), h0(1:17)]
    #   even t: h0(t-1)@rows 64:80, x(t)@row 48
    #   odd  t: h0(t-1)@rows 32:48, x(t)@row 49
    we0a, b0 = _prep_weights(enc_w0, enc_b0, 1 + C0, C0,
                             ZR * 48 + [0] + ZR * 15 +
                             list(range(1, 17)), 112)
    we0b, _ = _prep_weights(enc_w0, enc_b0, 1 + C0, C0,
                            ZR * 32 + list(range(1, 17)) + ZR + [0] +
                            ZR * 30, 112)
    # enc1(t): input ch = [h0(0:16), h1(16:48)]; h1@rows 0:32,
    #   h0(t)@rows 32:48 (even t) or 64:80 (odd t)
    we1a, b1 = _prep_weights(enc_w1, enc_b1, C0 + C1, C1,
                             list(range(16, 48)) + list(range(16)) +
                             ZR * 32, 128)
    we1b, _ = _prep_weights(enc_w1, enc_b1, C0 + C1, C1,
                            list(range(16, 48)) + ZR * 32 +
                            list(range(16)), 128)
    # dec0(t): input ch = [e2(0:32), hd0(32:64)]; e2@rows 96:128,
    #   hd0(t-1)@rows 64:96 (even t) or 32:64 (odd t)
    wd0a, bd0 = _prep_weights(dec_w0, dec_b0, C1 + C1, C1,
                              ZR * 64 + list(range(32, 64)) +
                              list(range(32)), 128)
    wd0b, _ = _prep_weights(dec_w0, dec_b0, C1 + C1, C1,
                            ZR * 32 + list(range(32, 64)) + ZR * 32 +
                            list(range(32)), 128)
    # dec1(t): input ch = [hd0(0:32), hd1(32:48)]; hd1@rows 0:16,
    #   hd0(t)@rows 32:64 (even t) or 64:96 (odd t)
    wd1a, bd1 = _prep_weights(dec_w1, dec_b1, C1 + C0, C0,
                              list(range(32, 48)) + ZR * 16 +
                              list(range(32)) + ZR * 32, 112)
    wd1b, _ = _prep_weights(dec_w1, dec_b1, C1 + C0, C0,
                            list(range(32, 48)) + ZR * 16 + ZR * 32 +
                            list(range(32)), 112)
    fcv = np.concatenate(
        [np.asarray(fc_w, np.float32).reshape(C0),
         np.asarray(fc_b, np.float32).reshape(1)]).reshape(17, 1)
    fcv = np.ascontiguousarray(fcv).astype(NP_BF16)

    in_maps = []
    for core in range(NCORES):
        xs = x[core * B:(core + 1) * B]      # [B, T, F]
        xp = np.zeros((T, B, SEG), np.float32)
        xp[:, :, 1:1 + F] = xs.transpose(1, 0, 2)
        in_maps.append({
            "x_pad": xp.astype(NP_BF16),
            "we0a": we0a, "we0b": we0b, "we1a": we1a, "we1b": we1b,
            "wd0a": wd0a, "wd0b": wd0b, "wd1a": wd1a, "wd1b": wd1b,
            "fcv": fcv,
            "b0": b0, "b1": b1, "bd0": bd0, "bd1": bd1,
        })

    _CACHE["in_maps"] = in_maps
    res = run_bass_kernel_spmd(nc, in_maps, core_ids=list(range(NCORES)))
    outs = [res.results[i]["out"] for i in range(NCORES)]
    return np.concatenate(outs, axis=0).astype(np.float32)


if __name__ == "__main__":
    rng = np.random.default_rng(0)
    inputs = {
        "x": rng.standard_normal((B_TOT, T, F), dtype=np.float32),
        "enc_w0": rng.standard_normal((4 * C0, 1 + C0, 3, 3), dtype=np.float32) * 0.05,
        "enc_b0": np.zeros(4 * C0, np.float32),
        "enc_w1": rng.standard_normal((4 * C1, C0 + C1, 3, 3), dtype=np.float32) * 0.05,
        "enc_b1": np.zeros(4 * C1, np.float32),
        "dec_w0": rng.standard_normal((4 * C1, C1 + C1, 3, 3), dtype=np.float32) * 0.05,
        "dec_b0": np.zeros(4 * C1, np.float32),
        "dec_w1": rng.standard_normal((4 * C0, C1 + C0, 3, 3), dtype=np.float32) * 0.05,
        "dec_b1": np.zeros(4 * C0, np.float32),
        "fc_w": rng.standard_normal((1, C0, 1, 1), dtype=np.float32) * 0.05,
        "fc_b": np.zeros(1, np.float32),
    }
    out = kernel(**inputs)
    print("out", out.shape, out.dtype, np.abs(out).max())
